# revision 1
# baseline (speedup 1.0000x reference)
"""EulerAttention Trainium2 kernel.

Per-core sharding: core c in 0..7 -> (batch b = c // 4, query block qb = c % 4,
1024 queries each).  Each core computes K/V (+ feature maps) for its whole
batch, Q features for its query block, then flash-style scores/softmax/AV.

All matmuls run as float32r (full-rate fp32 with reduced multiplier mantissa);
e-tile 0 of the Q/K projections runs in full fp32 (the 1/wavelength scaling
amplifies its error ~200x more than the rest).  Feature maps cos/sin(theta)
use a turns-space range reduction (magic-number round + add_range_wrap)
feeding the ACT Sin table (valid +-pi).  Softmax runs without max-subtraction
(logits are bounded by sqrt(D)), rowsums via ones-matmul, normalization and
the V-bias fold happen on the output tiles.

kernel(**inputs) takes the full unsharded inputs from reference.setup_inputs()
and returns the full [B, S, D] output.
"""
import sys, math

sys.path.insert(0, "/opt/trn_rl_repo")

import numpy as np

B, S, D = 2, 4096, 1024
NCORES = 8
QBLK = S // 4          # queries per core
ET = D // 128          # number of 128-row e/d tiles (8)
MAGIC = float(1.5 * 2**23)
TWOPI = 2.0 * math.pi
INV_SQRT_D = 1.0 / math.sqrt(D)

_cache = {}


def _build_program(s_keys=S, s_q=QBLK, trace_sim=False, fp32_et0=True):
    """Build the SPMD bass program. s_keys/s_q parameterizable for mini-tests."""
    import concourse.bass as bass
    from concourse import bacc
    import concourse.mybir as mybir
    import concourse.tile as tile
    from contextlib import ExitStack

    f32 = mybir.dt.float32
    f32r = mybir.dt.float32r
    Act = mybir.ActivationFunctionType
    Alu = mybir.AluOpType

    n_sblk = s_keys // 512       # key production blocks (4 t-tiles each)
    n_tt = s_keys // 128         # key tiles (t)
    n_tgrp = max(1, n_tt // 8)   # AV groups of 8 t-tiles
    tt_per_grp = n_tt // n_tgrp
    n_qsb = s_q // 512           # query production blocks
    NS = s_q                     # resident query width (free dim in phase 2)
    n_ns = NS // 512             # N-splits for matmuls over queries
    n_eg = ET // 2               # et store groups of 2

    nc = bacc.Bacc("TRN2", target_bir_lowering=False, debug=False)

    xT = nc.dram_tensor("xT", [D, s_keys], f32, kind="ExternalInput").ap()
    xTq = nc.dram_tensor("xTq", [D, s_q], f32, kind="ExternalInput").ap()
    Wq0 = nc.dram_tensor("Wq0", [D, 128], f32, kind="ExternalInput").ap()
    Wk0 = nc.dram_tensor("Wk0", [D, 128], f32, kind="ExternalInput").ap()
    WqT = nc.dram_tensor("WqT", [D, D], f32r, kind="ExternalInput").ap()
    WkT = nc.dram_tensor("WkT", [D, D], f32r, kind="ExternalInput").ap()
    WvT = nc.dram_tensor("WvT", [D, D], f32r, kind="ExternalInput").ap()
    # packed per-partition constants: columns = (sc2 | bq2 | bk2 | bv) x ET
    CON = nc.dram_tensor("CON", [128, 4 * ET], f32, kind="ExternalInput").ap()

    OT = nc.dram_tensor("OT", [D, s_q], f32, kind="ExternalOutput").ap()

    with tile.TileContext(nc, trace_sim=trace_sim) as tc, ExitStack() as top:
        # ---- DRAM intermediates, split per block for fine-grained RAW deps ----
        dram = top.enter_context(tc.tile_pool(name="dram", bufs=1, space="DRAM"))
        CK_d = [dram.tile([D, 512], f32r, tag=f"ck{i}", name=f"ckd{i}")
                for i in range(n_sblk)]
        SK_d = [dram.tile([D, 512], f32r, tag=f"sk{i}", name=f"skd{i}")
                for i in range(n_sblk)]
        V_d = [dram.tile([512, D], f32r, tag=f"v{i}", name=f"vd{i}")
               for i in range(n_sblk)]

        # ---- constants (tiny, load first) ----
        cpool = top.enter_context(tc.tile_pool(name="consts", bufs=1))
        ctile = cpool.tile([128, 4 * ET], f32, tag="ctile")
        nc.sync.dma_start(ctile[:], CON[:])
        sc2 = [ctile[:, i : i + 1] for i in range(ET)]
        bq2 = [ctile[:, ET + i : ET + i + 1] for i in range(ET)]
        bk2 = [ctile[:, 2 * ET + i : 2 * ET + i + 1] for i in range(ET)]
        bvt = [ctile[:, 3 * ET + i : 3 * ET + i + 1] for i in range(ET)]
        ones_f = cpool.tile([128, 2], f32, tag="ones_f")
        nc.vector.memset(ones_f[:], 1.0)
        ones_col = cpool.tile([128, 2], f32r, tag="ones_col")  # [K=128, M=2] rowsum lhsT
        nc.vector.tensor_copy(ones_col[:], ones_f[:])
        ones_rf = cpool.tile([1, 128], f32, tag="ones_rf")
        nc.vector.memset(ones_rf[:], 1.0)
        ones_row = cpool.tile([1, 128], f32r, tag="ones_row")  # [K=1, M=128] bcast lhsT
        nc.vector.tensor_copy(ones_row[:], ones_rf[:])

        # ---- shared PSUM pool: proj (1 bank x2), big (2 banks x2), rs (2) ----
        psum = top.enter_context(tc.tile_pool(name="psum", bufs=1, space="PSUM"))

        # ---- resident Q feature maps, layout [128, (et, qsb, 512)]; written
        # directly by the Q-feature ACT ops, consumed by phase-2 matmuls ----
        qres = top.enter_context(tc.tile_pool(name="qres", bufs=1))
        cqa = qres.tile([128, ET * NS], f32r, tag="cqa")
        sqa = qres.tile([128, ET * NS], f32r, tag="sqa")

        # ---- weights: wq and wv share tiles (wq used only in the Q section);
        # fp32 copies of the e-tile-0 weight columns for the precise matmuls ----
        wpool_ctx = tc.tile_pool(name="w", bufs=1)
        wpool = wpool_ctx.__enter__()
        wsh = [wpool.tile([128, D], f32r, tag=f"wsh{d}", name=f"wsh{d}") for d in range(ET)]
        wk = [wpool.tile([128, D], f32r, tag=f"wk{d}", name=f"wk{d}") for d in range(ET)]
        w0 = [wpool.tile([128, 128], f32, tag=f"w0{d}", name=f"w0{d}") for d in range(ET)]
        if fp32_et0:
            for d in range(ET):
                nc.sync.dma_start(w0[d][:], Wq0[d * 128 : (d + 1) * 128, :])

        # ================= PHASE 1: projections + feature maps =================
        with tc.tile_pool(name="p1sb", bufs=2) as p1, \
             tc.tile_pool(name="p1chain", bufs=2) as pch:
            pps = psum

            def load_xblk(src_ap, col0):
                """One DMA: [1024, 512] dram slice -> fp32 block; DVE makes the
                rounded f32r copy for the fast-path matmuls (the DMA itself
                rounds when writing f32r, so the fp32 load preserves the full
                data for the e-tile-0 fp32 matmuls)."""
                b32 = p1.tile([128, ET * 512], f32, tag="xb32", name="xb32", bufs=1)
                nc.sync.dma_start(
                    b32[:].rearrange("p (d s) -> p d s", d=ET),
                    src_ap[:, col0 : col0 + 512].rearrange("(d p) s -> p d s", p=128))
                br = p1.tile([128, ET * 512], f32r, tag="xbr", name="xbr")
                nc.vector.tensor_copy(br[:], b32[:])
                return b32, br

            def feature_block(xb, et, w_tiles, bias_tiles, c_stage, s_stage):
                """Produce cos/sin feature tiles [128, 512] (f32r) for one e-tile."""
                xb32, xbr = xb
                ps = pps.tile([128, 512], f32, tag="proj", name="psf", bufs=2)
                for d in range(ET):
                    if fp32_et0 and et == 0:
                        lhs = w0[d][:]
                        rhs = xb32[:, d * 512 : (d + 1) * 512]
                    else:
                        lhs = w_tiles[d][:, et * 128 : (et + 1) * 128]
                        rhs = xbr[:, d * 512 : (d + 1) * 512]
                    nc.tensor.matmul(ps[:], lhs, rhs,
                                     start=(d == 0), stop=(d == ET - 1))
                r = pch.tile([128, 512], f32, tag="r", name="r")
                nc.scalar.activation(r[:], ps[:], Act.Identity,
                                     scale=sc2[et][:], bias=bias_tiles[et][:])
                kk = pch.tile([128, 512], f32, tag="kk", name="kk")
                nc.vector.tensor_scalar(kk[:], r[:], MAGIC, MAGIC, Alu.add, Alu.subtract)
                f = pch.tile([128, 512], f32, tag="f", name="f")
                nc.vector.scalar_tensor_tensor(f[:], kk[:], -1.0, r[:],
                                               Alu.mult, Alu.add)
                nc.scalar.activation(s_stage[:], f[:], Act.Sin, scale=TWOPI)
                g = pch.tile([128, 512], f32, tag="kk", name="g")
                nc.vector.add_range_wrap(g[:], f[:], 0.25, 0.5, 1.0)
                nc.scalar.activation(c_stage[:], g[:], Act.Sin, scale=TWOPI)

            def emit_k_features(xb, cdst, sdst):
                """K features: ACT output tiles stored directly per e-tile."""
                for et in range(ET):
                    cst = pch.tile([128, 512], f32r, tag="cst", name="cst")
                    sst = pch.tile([128, 512], f32r, tag="sst", name="sst")
                    feature_block(xb, et, wk, bk2, cst[:], sst[:])
                    nc.sync.dma_start(cdst[et * 128 : (et + 1) * 128, :], cst[:])
                    nc.sync.dma_start(sdst[et * 128 : (et + 1) * 128, :], sst[:])

            # --- Q features, written straight into the resident cqa/sqa ---
            xq_blocks = [load_xblk(xTq, 0)]
            for d in range(ET):
                nc.sync.dma_start(wsh[d][:], WqT[d * 128 : (d + 1) * 128, :])
            if n_qsb > 1:
                xq_blocks.append(load_xblk(xTq, 512))
            for d in range(ET):
                nc.sync.dma_start(wk[d][:], WkT[d * 128 : (d + 1) * 128, :])
            for qsb in range(n_qsb):
                xqb = xq_blocks[qsb]
                for et in range(ET):
                    feature_block(
                        xqb, et, wsh, bq2,
                        cqa[:, et * NS + qsb * 512 : et * NS + qsb * 512 + 512],
                        sqa[:, et * NS + qsb * 512 : et * NS + qsb * 512 + 512])

            if fp32_et0:
                for d in range(ET):
                    # w0k overwrites w0q (WAR dep handled by Tile)
                    nc.sync.dma_start(w0[d][:], Wk0[d * 128 : (d + 1) * 128, :])
            for d in range(ET):
                # wv overwrites the wq tiles (WAR dep handled by Tile)
                nc.sync.dma_start(wsh[d][:], WvT[d * 128 : (d + 1) * 128, :])

            # --- K features + V ---
            for sblk in range(n_sblk):
                xkb = load_xblk(xT, sblk * 512)
                emit_k_features(xkb, CK_d[sblk], SK_d[sblk])
                # V in natural [t, dv] layout, no bias (folded into output)
                for ti in range(4):
                    for dg in range(2):
                        psv = pps.tile([128, 512], f32, tag="proj", name="psv", bufs=2)
                        for d in range(ET):
                            nc.tensor.matmul(
                                psv[:], xkb[1][:, d * 512 + ti * 128 : d * 512 + (ti + 1) * 128],
                                wsh[d][:, dg * 512 : dg * 512 + 512],
                                start=(d == 0), stop=(d == ET - 1))
                        vstg = p1.tile([128, 512], f32r, tag="vstg", name="vstg")
                        nc.vector.tensor_copy(vstg[:], psv[:])
                        nc.sync.dma_start(
                            V_d[sblk][ti * 128 : (ti + 1) * 128,
                                      dg * 512 : (dg + 1) * 512], vstg[:])

        wpool_ctx.__exit__(None, None, None)

        # ================= PHASE 2: scores + softmax + AV =================
        with tc.tile_pool(name="p2sb", bufs=2) as p2, \
             tc.tile_pool(name="epool", bufs=tt_per_grp + 1) as epool, \
             tc.tile_pool(name="vpool", bufs=8) as vpool, \
             tc.tile_pool(name="oacc", bufs=1) as oacc:
            p2ps = psum
            rsps = psum

            def qslice(big, et, ns):
                return big[:, et * NS + ns * 512 : et * NS + ns * 512 + 512]

            o_ac = [oacc.tile([128, NS], f32, tag=f"o{dt}", name=f"oac{dt}")
                    for dt in range(ET)]
            ps_rs = rsps.tile([2, NS], f32, tag="rs", bufs=1)

            for tg in range(n_tgrp):
                e_tiles = []
                for ti in range(tt_per_grp):
                    tt = tg * tt_per_grp + ti
                    sb_i, loc = tt // 4, tt % 4
                    ck = p2.tile([128, D], f32r, tag="ck", name="ck")
                    sk = p2.tile([128, D], f32r, tag="sk", name="sk")
                    for dst, src in ((ck, CK_d[sb_i]), (sk, SK_d[sb_i])):
                        nc.sync.dma_start(
                            dst[:].rearrange("p (et t) -> p et t", et=ET),
                            src[:, loc * 128 : (loc + 1) * 128]
                            .rearrange("(et p) t -> p et t", p=128))
                    ps_sim = p2ps.tile([128, NS], f32, tag="big", name="ps_sim", bufs=2)
                    for ns in range(n_ns):
                        sl = slice(ns * 512, ns * 512 + 512)
                        for et in range(ET):
                            nc.tensor.matmul(ps_sim[:, sl],
                                             ck[:, et * 128 : (et + 1) * 128],
                                             qslice(cqa, et, ns),
                                             start=(et == 0), stop=False)
                        for et in range(ET):
                            nc.tensor.matmul(ps_sim[:, sl],
                                             sk[:, et * 128 : (et + 1) * 128],
                                             qslice(sqa, et, ns),
                                             start=False, stop=(et == ET - 1))
                    et_t = epool.tile([128, NS], f32r, tag="e", name="e")
                    nc.scalar.activation(et_t[:], ps_sim[:], Act.Exp, scale=INV_SQRT_D)
                    e_tiles.append((tt, et_t))
                    for ns in range(n_ns):
                        sl = slice(ns * 512, ns * 512 + 512)
                        nc.tensor.matmul(ps_rs[:, sl], ones_col[:], et_t[:, sl],
                                         start=(tt == 0), stop=(tt == n_tt - 1))
                # AV for this group
                for dg in range(2):
                    vts = []
                    for gi, (tt, _) in enumerate(e_tiles):
                        sb_i, loc = tt // 4, tt % 4
                        vt = vpool.tile([128, 512], f32r, tag="vt", name="vt")
                        nc.sync.dma_start(
                            vt[:], V_d[sb_i][loc * 128 : (loc + 1) * 128,
                                             dg * 512 : (dg + 1) * 512])
                        vts.append(vt)
                    for di in range(4):
                        dt = dg * 4 + di
                        ps_o = p2ps.tile([128, NS], f32, tag="big", name="ps_o", bufs=2)
                        for gi, (tt, et_t) in enumerate(e_tiles):
                            for ns in range(n_ns):
                                sl = slice(ns * 512, ns * 512 + 512)
                                nc.tensor.matmul(
                                    ps_o[:, sl], vts[gi][:, di * 128 : (di + 1) * 128],
                                    et_t[:, sl],
                                    start=(gi == 0), stop=(gi == len(e_tiles) - 1))
                        if tg == 0:
                            nc.vector.tensor_copy(o_ac[dt][:], ps_o[:])
                        else:
                            nc.vector.tensor_tensor(o_ac[dt][:], ps_o[:], o_ac[dt][:],
                                                    Alu.add)

            # normalize: recip of rowsum, broadcast via rank-1 matmul; + V bias
            rs_sb = p2.tile([1, NS], f32, tag="rs_sb")
            nc.vector.tensor_copy(rs_sb[:], ps_rs[:1, :])
            rec_f = p2.tile([1, NS], f32, tag="rec_f")
            nc.vector.reciprocal(rec_f[:], rs_sb[:])
            rec = p2.tile([1, NS], f32r, tag="rec")
            nc.vector.tensor_copy(rec[:], rec_f[:])
            ps_bc = p2ps.tile([128, NS], f32, tag="big", name="ps_bc", bufs=2)
            for ns in range(n_ns):
                sl = slice(ns * 512, ns * 512 + 512)
                nc.tensor.matmul(ps_bc[:, sl], ones_row[:], rec[:, sl],
                                 start=True, stop=True)
            bc = p2.tile([128, NS], f32, tag="bc")
            nc.vector.tensor_copy(bc[:], ps_bc[:])
            for dt in range(ET):
                on = p2.tile([128, NS], f32, tag="on", name="on")
                nc.vector.tensor_tensor(on[:], o_ac[dt][:], bc[:], Alu.mult)
                # per-partition V-bias add on ACT (idle at the tail)
                nc.scalar.activation(on[:], on[:], Act.Identity, bias=bvt[dt][:])
                nc.sync.dma_start(OT[dt * 128 : (dt + 1) * 128, :], on[:])

    nc.compile()
    return nc


def _host_prep(x, Wq, bq, Wk, bk, Wv, bv, phase_bias):
    wavelengths = np.arange(1, D + 1, dtype=np.float32) * np.float32(2.0 * math.pi / D)
    inv_wl = (np.float32(1.0) / (wavelengths + np.float32(1e-8))).astype(np.float32)
    sc2 = (inv_wl / TWOPI).astype(np.float32).reshape(ET, 128)
    bq2 = ((bq * inv_wl + phase_bias) / TWOPI).astype(np.float32).reshape(ET, 128)
    bk2 = ((bk * inv_wl + phase_bias) / TWOPI).astype(np.float32).reshape(ET, 128)
    WqT = np.ascontiguousarray(Wq.T).astype(np.float32)
    WkT = np.ascontiguousarray(Wk.T).astype(np.float32)
    WvT = np.ascontiguousarray(Wv.T).astype(np.float32)
    xT = [np.ascontiguousarray(x[b].T).astype(np.float32) for b in range(x.shape[0])]
    con = np.stack([sc2, bq2, bk2, bv.reshape(ET, 128).astype(np.float32)])
    # [4, ET, 128] -> [128, 4*ET] with column layout (kind, et)
    con = np.ascontiguousarray(con.reshape(4 * ET, 128).T).astype(np.float32)
    return xT, WqT, WkT, WvT, con


def kernel(x, Wq, bq, Wk, bk, Wv, bv, phase_bias, _trace=False):
    from concourse.bass_utils import run_bass_kernel_spmd

    x = np.asarray(x, dtype=np.float32)
    xT, WqT, WkT, WvT, con = _host_prep(
        x, np.asarray(Wq, np.float32), np.asarray(bq, np.float32),
        np.asarray(Wk, np.float32), np.asarray(bk, np.float32),
        np.asarray(Wv, np.float32), np.asarray(bv, np.float32),
        np.asarray(phase_bias, np.float32))

    if "prog" not in _cache:
        _cache["prog"] = _build_program()
    nc = _cache["prog"]

    in_maps = []
    for c in range(NCORES):
        b, qb = c // 4, c % 4
        in_maps.append({
            "xT": xT[b],
            "xTq": np.ascontiguousarray(xT[b][:, qb * QBLK : (qb + 1) * QBLK]),
            "WqT": WqT, "WkT": WkT, "WvT": WvT,
            "Wq0": np.ascontiguousarray(WqT[:, :128]),
            "Wk0": np.ascontiguousarray(WkT[:, :128]),
            "CON": con,
        })
    res = run_bass_kernel_spmd(nc, in_maps, core_ids=list(range(NCORES)),
                               trace=_trace)
    out = np.empty((B, S, D), dtype=np.float32)
    for c in range(NCORES):
        b, qb = c // 4, c % 4
        out[b, qb * QBLK : (qb + 1) * QBLK, :] = res.results[c]["OT"].T
    if _trace:
        kernel.last_exec_time_ns = res.exec_time_ns
        kernel.last_result = res
    return out



# revision 16
# speedup vs baseline: 1.4403x; 1.4403x over previous
"""EulerAttention Trainium2 kernel (v2: fp8 DoubleRow scores).

Per-core sharding: core c in 0..7 -> (batch b = c // 4, query block qb = c % 4,
1024 queries each).  Each core computes K features for its whole batch, Q
features for its query block, then flash-style scores/softmax/AV.

v2 design:
- All projections f32r (full-rate).  Q/K features (cos/sin theta) are stored
  as fp8e4 and the [S,S] score matmuls run in fp8 DoubleRow perf mode (2
  128-feature tiles contracted per pass, 0.5 cyc/row -> 4x f32r).
- K-side features are mean-centered per feature (host-computed E[cos theta_k],
  E[sin theta_k] from the weights); the dropped cross terms are per-query
  constants that cancel in softmax normalization.  Centering shrinks fp8
  quantization noise from the near-constant long-wavelength features.
- K features stay SBUF-resident (fp8, 64KB/partition); no DRAM roundtrip.
- V and the exp(scores) tiles are bf16; AV + rowsum matmuls run bf16.
  V stays SBUF-resident per 2-sblk AV group.  Output accumulator bf16.
- Two sweeps: sweep 1 = Q features + K features (wq/wk resident);
  sweep 2 = V proj + scores + softmax + AV (wv resident, x reloaded).

kernel(**inputs) takes the full unsharded inputs from reference.setup_inputs()
and returns the full [B, S, D] output.
"""
import sys, math

sys.path.insert(0, "/opt/trn_rl_repo")

import numpy as np
import ml_dtypes

B, S, D = 2, 4096, 1024
NCORES = 8
QBLK = S // 4          # queries per core
ET = D // 128          # number of 128-row e/d tiles (8)
NSLOT = 2 * ET         # fp8 feature slots (cos/sin interleaved per et)
MAGIC = float(1.5 * 2**23)
TWOPI = 2.0 * math.pi
INV_SQRT_D = 1.0 / math.sqrt(D)
F8NP = ml_dtypes.float8_e4m3

_cache = {}


def _build_program(trace_sim=False):
    import concourse.bass as bass
    from concourse import bacc
    import concourse.mybir as mybir
    import concourse.tile as tile
    from contextlib import ExitStack

    f32 = mybir.dt.float32
    f32r = mybir.dt.float32r
    bf16 = mybir.dt.bfloat16
    f8 = mybir.dt.float8e4
    Act = mybir.ActivationFunctionType
    Alu = mybir.AluOpType
    PM = mybir.MatmulPerfMode

    s_keys, s_q = S, QBLK
    n_sblk = s_keys // 512       # 8 key production blocks
    n_tt = s_keys // 128         # 32 key tiles
    n_qsb = s_q // 512           # 2 query production blocks
    NS = s_q                     # resident query width (1024)
    n_ns = NS // 512             # N-splits for matmuls over queries

    nc = bacc.Bacc("TRN2", target_bir_lowering=False, debug=False)

    xT = nc.dram_tensor("xT", [D, s_keys], f32r, kind="ExternalInput").ap()
    xTq = nc.dram_tensor("xTq", [D, s_q], f32r, kind="ExternalInput").ap()
    WqT = nc.dram_tensor("WqT", [D, D], f32r, kind="ExternalInput").ap()
    WkT = nc.dram_tensor("WkT", [D, D], f32r, kind="ExternalInput").ap()
    WvT = nc.dram_tensor("WvT", [D, D], f32r, kind="ExternalInput").ap()
    # packed per-partition constants: columns = (sc2 | bq2 | bk2 | bv | nac | nas) x ET
    CON = nc.dram_tensor("CON", [128, 6 * ET], f32, kind="ExternalInput").ap()

    OT = nc.dram_tensor("OT", [D, s_q], f32, kind="ExternalOutput").ap()

    with tile.TileContext(nc, trace_sim=trace_sim) as tc, ExitStack() as top:
        # ---- constants (tiny, load first) ----
        cpool = top.enter_context(tc.tile_pool(name="consts", bufs=1))
        ctile = cpool.tile([128, 6 * ET], f32, tag="ctile")
        nc.sync.dma_start(ctile[:], CON[:])
        sc2 = [ctile[:, i : i + 1] for i in range(ET)]
        bq2 = [ctile[:, ET + i : ET + i + 1] for i in range(ET)]
        bk2 = [ctile[:, 2 * ET + i : 2 * ET + i + 1] for i in range(ET)]
        bvt = [ctile[:, 3 * ET + i : 3 * ET + i + 1] for i in range(ET)]
        nac = [ctile[:, 4 * ET + i : 4 * ET + i + 1] for i in range(ET)]
        nas = [ctile[:, 5 * ET + i : 5 * ET + i + 1] for i in range(ET)]
        ones_col = cpool.tile([128, 2], bf16, tag="ones_col")  # [K=128, M=2] rowsum lhsT
        nc.vector.memset(ones_col[:], 1.0)
        ones_rf = cpool.tile([1, 128], f32, tag="ones_rf")
        nc.vector.memset(ones_rf[:], 1.0)
        ones_row = cpool.tile([1, 128], f32r, tag="ones_row")  # [K=1, M=128] bcast lhsT
        nc.vector.tensor_copy(ones_row[:], ones_rf[:])

        # ---- shared PSUM pool ----
        psum = top.enter_context(tc.tile_pool(name="psum", bufs=1, space="PSUM"))

        # ---- output accumulator (bf16), lives until the final normalize ----
        oacc = top.enter_context(tc.tile_pool(name="oacc", bufs=1))
        o_ac = [oacc.tile([128, NS], bf16, tag=f"o{dt}", name=f"oac{dt}")
                for dt in range(ET)]

        # ---- resident fp8 feature maps ----
        # Q: [128, slot, q]; K: per sblk [128, slot, 512 keys]
        qres_ctx = tc.tile_pool(name="qres", bufs=1)
        qres = qres_ctx.__enter__()
        qa8 = qres.tile([128, NSLOT, NS], f8, tag="qa8")

        kpool_ctx = tc.tile_pool(name="kres", bufs=1)
        kpool = kpool_ctx.__enter__()
        kres = [kpool.tile([128, NSLOT, 512], f8, tag=f"kr{s}", name=f"kr{s}")
                for s in range(n_sblk)]

        def qslot(et, cs, qsb):
            return qa8[:, 2 * et + cs, qsb * 512 : qsb * 512 + 512]

        # ================= SWEEP 1: Q + K feature maps =================
        p1_ctx = tc.tile_pool(name="p1sb", bufs=2)
        p1 = p1_ctx.__enter__()
        w1_ctx = tc.tile_pool(name="w1", bufs=1)
        w1 = w1_ctx.__enter__()
        pch_ctx = tc.tile_pool(name="p1chain", bufs=2)
        pch = pch_ctx.__enter__()

        # one shared weight set: wq for the Q section, then overwritten by wk
        wsh = [w1.tile([128, D], f32r, tag=f"w{d}", name=f"w{d}") for d in range(ET)]

        def load_xblk(src_ap, col0):
            br = p1.tile([128, ET * 512], f32r, tag="xbr", name="xbr")
            nc.sync.dma_start(
                br[:].rearrange("p (d s) -> p d s", d=ET),
                src_ap[:, col0 : col0 + 512].rearrange("(d p) s -> p d s", p=128))
            return br

        def theta_chain(xb, et, w_tiles, bias_tiles):
            """Projection + range reduction; returns (f, g) turn tiles for Sin."""
            ps = psum.tile([128, 512], f32, tag="proj", name="psf", bufs=2)
            for d in range(ET):
                nc.tensor.matmul(ps[:], w_tiles[d][:, et * 128 : (et + 1) * 128],
                                 xb[:, d * 512 : (d + 1) * 512],
                                 start=(d == 0), stop=(d == ET - 1))
            r = pch.tile([128, 512], f32, tag="r", name="r")
            nc.scalar.activation(r[:], ps[:], Act.Identity,
                                 scale=sc2[et][:], bias=bias_tiles[et][:])
            kk = pch.tile([128, 512], f32, tag="kk", name="kk")
            nc.vector.tensor_scalar(kk[:], r[:], MAGIC, MAGIC, Alu.add, Alu.subtract)
            f = pch.tile([128, 512], f32, tag="f", name="f")
            nc.vector.scalar_tensor_tensor(f[:], kk[:], -1.0, r[:],
                                           Alu.mult, Alu.add)
            g = pch.tile([128, 512], f32, tag="kk", name="g")
            nc.vector.add_range_wrap(g[:], f[:], 0.25, 0.5, 1.0)
            return f, g

        # --- Q features: ACT Sin writes fp8 directly into qa8 ---
        xq_blocks = [load_xblk(xTq, 0)]
        for d in range(ET):
            nc.sync.dma_start(wsh[d][:], WqT[d * 128 : (d + 1) * 128, :])
        if n_qsb > 1:
            xq_blocks.append(load_xblk(xTq, 512))
        for qsb in range(n_qsb):
            for et in range(ET):
                f, g = theta_chain(xq_blocks[qsb], et, wsh, bq2)
                nc.scalar.activation(qslot(et, 1, qsb), f[:], Act.Sin, scale=TWOPI)
                nc.scalar.activation(qslot(et, 0, qsb), g[:], Act.Sin, scale=TWOPI)

        # --- K features: ACT Sin -> f32, Pool centers + converts to fp8 ---
        for d in range(ET):
            # wk overwrites wq (WAR dep handled by Tile)
            nc.sync.dma_start(wsh[d][:], WkT[d * 128 : (d + 1) * 128, :])
        for sblk in range(n_sblk):
            xkb = load_xblk(xT, sblk * 512)
            for et in range(ET):
                f, g = theta_chain(xkb, et, wsh, bk2)
                s32 = pch.tile([128, 512], f32, tag="s32", name="s32")
                nc.scalar.activation(s32[:], f[:], Act.Sin, scale=TWOPI)
                nc.gpsimd.tensor_scalar(kres[sblk][:, 2 * et + 1, :], s32[:],
                                        nas[et][:], None, Alu.add)
                c32 = pch.tile([128, 512], f32, tag="s32", name="c32")
                nc.scalar.activation(c32[:], g[:], Act.Sin, scale=TWOPI)
                nc.gpsimd.tensor_scalar(kres[sblk][:, 2 * et, :], c32[:],
                                        nac[et][:], None, Alu.add)

        pch_ctx.__exit__(None, None, None)
        w1_ctx.__exit__(None, None, None)

        # ================= SWEEP 2: V proj + scores + softmax + AV =========
        with tc.tile_pool(name="w2", bufs=1) as w2, \
             tc.tile_pool(name="vres", bufs=1) as vpool, \
             tc.tile_pool(name="epool", bufs=9) as epool:

            wv = [w2.tile([128, D], f32r, tag=f"wv{d}", name=f"wv{d}")
                  for d in range(ET)]
            for d in range(ET):
                nc.sync.dma_start(wv[d][:], WvT[d * 128 : (d + 1) * 128, :])

            # V resident per 2-sblk AV group: [128 p=key%128, ti, dv]
            vres = [vpool.tile([128, 4, D], bf16, tag=f"v{i}", name=f"v{i}")
                    for i in range(2)]
            ps_rs = psum.tile([2, NS], f32, tag="rs", bufs=1)

            e_group = []
            for sblk in range(n_sblk):
                xkb = load_xblk(xT, sblk * 512)
                # V projection for this key block
                for ti in range(4):
                    for dg in range(2):
                        psv = psum.tile([128, 512], f32, tag="proj", name="psv",
                                        bufs=2)
                        for d in range(ET):
                            nc.tensor.matmul(
                                psv[:],
                                xkb[:, d * 512 + ti * 128 : d * 512 + (ti + 1) * 128],
                                wv[d][:, dg * 512 : dg * 512 + 512],
                                start=(d == 0), stop=(d == ET - 1))
                        nc.vector.tensor_copy(
                            vres[sblk % 2][:, ti, dg * 512 : (dg + 1) * 512], psv[:])
                # scores + exp for the 4 key tiles of this block
                for loc in range(4):
                    tt = sblk * 4 + loc
                    ps_sim = psum.tile([128, NS], f32, tag="big", name="ps_sim",
                                       bufs=2)
                    for ns in range(n_ns):
                        sl = slice(ns * 512, ns * 512 + 512)
                        for j in range(ET):
                            nc.tensor.matmul(
                                ps_sim[:, sl],
                                kres[sblk][:, 2 * j : 2 * j + 2,
                                           loc * 128 : (loc + 1) * 128],
                                qa8[:, 2 * j : 2 * j + 2, ns * 512 : ns * 512 + 512],
                                start=(j == 0), stop=(j == ET - 1),
                                perf_mode=PM.DoubleRow)
                    et_t = epool.tile([128, NS], bf16, tag="e", name="e")
                    nc.scalar.activation(et_t[:], ps_sim[:], Act.Exp,
                                         scale=INV_SQRT_D)
                    e_group.append(et_t)
                    for ns in range(n_ns):
                        sl = slice(ns * 512, ns * 512 + 512)
                        nc.tensor.matmul(ps_rs[:, sl], ones_col[:], et_t[:, sl],
                                         start=(tt == 0), stop=(tt == n_tt - 1))
                # AV for the group of 8 key tiles (2 sblks)
                if sblk % 2 == 1:
                    tg = sblk // 2
                    for dg in range(2):
                        for di in range(4):
                            dt = dg * 4 + di
                            ps_o = psum.tile([128, NS], f32, tag="big", name="ps_o",
                                             bufs=2)
                            for gi in range(8):
                                g_s, ti = gi // 4, gi % 4
                                for ns in range(n_ns):
                                    sl = slice(ns * 512, ns * 512 + 512)
                                    nc.tensor.matmul(
                                        ps_o[:, sl],
                                        vres[g_s][:, ti,
                                                  dt * 128 : (dt + 1) * 128],
                                        e_group[gi][:, sl],
                                        start=(gi == 0), stop=(gi == 7))
                            if tg == 0:
                                nc.vector.tensor_copy(o_ac[dt][:], ps_o[:])
                            else:
                                nc.vector.tensor_tensor(o_ac[dt][:], ps_o[:],
                                                        o_ac[dt][:], Alu.add)
                    e_group = []

        # release the big resident pools before the small normalize phase
        p1_ctx.__exit__(None, None, None)
        kpool_ctx.__exit__(None, None, None)
        qres_ctx.__exit__(None, None, None)

        # normalize: recip of rowsum, broadcast via rank-1 matmul; + V bias
        with tc.tile_pool(name="p2sb", bufs=1) as p2, \
             tc.tile_pool(name="pon", bufs=2) as pon:
            rs_sb = p2.tile([1, NS], f32, tag="rs_sb")
            nc.vector.tensor_copy(rs_sb[:], ps_rs[:1, :])
            rec_f = p2.tile([1, NS], f32, tag="rec_f")
            nc.vector.reciprocal(rec_f[:], rs_sb[:])
            rec = p2.tile([1, NS], f32r, tag="rec")
            nc.vector.tensor_copy(rec[:], rec_f[:])
            ps_bc = psum.tile([128, NS], f32, tag="big", name="ps_bc", bufs=2)
            for ns in range(n_ns):
                sl = slice(ns * 512, ns * 512 + 512)
                nc.tensor.matmul(ps_bc[:, sl], ones_row[:], rec[:, sl],
                                 start=True, stop=True)
            bc = p2.tile([128, NS], f32, tag="bc")
            nc.vector.tensor_copy(bc[:], ps_bc[:])
            for dt in range(ET):
                on = pon.tile([128, NS], f32, tag="on", name="on")
                nc.vector.tensor_tensor(on[:], o_ac[dt][:], bc[:], Alu.mult)
                nc.scalar.activation(on[:], on[:], Act.Identity, bias=bvt[dt][:])
                nc.sync.dma_start(OT[dt * 128 : (dt + 1) * 128, :], on[:])

    nc.compile()
    return nc


def _host_prep(x, Wq, bq, Wk, bk, Wv, bv, phase_bias):
    wavelengths = np.arange(1, D + 1, dtype=np.float32) * np.float32(2.0 * math.pi / D)
    inv_wl = (np.float32(1.0) / (wavelengths + np.float32(1e-8))).astype(np.float32)
    sc2 = (inv_wl / TWOPI).astype(np.float32).reshape(ET, 128)
    bq2 = ((bq * inv_wl + phase_bias) / TWOPI).astype(np.float32).reshape(ET, 128)
    bk2 = ((bk * inv_wl + phase_bias) / TWOPI).astype(np.float32).reshape(ET, 128)
    # K-feature means from the weights: theta_k ~ N(bk*ivl + pb, |wk_row|^2 ivl^2)
    mu = (bk * inv_wl + phase_bias).astype(np.float64)
    var = (np.sum(Wk.astype(np.float64) ** 2, axis=1) * inv_wl.astype(np.float64) ** 2)
    damp = np.exp(-var / 2.0)
    nac = (-(np.cos(mu) * damp)).astype(np.float32).reshape(ET, 128)
    nas = (-(np.sin(mu) * damp)).astype(np.float32).reshape(ET, 128)
    WqT = np.ascontiguousarray(Wq.T).astype(np.float32)
    WkT = np.ascontiguousarray(Wk.T).astype(np.float32)
    WvT = np.ascontiguousarray(Wv.T).astype(np.float32)
    xT = [np.ascontiguousarray(x[b].T).astype(np.float32) for b in range(x.shape[0])]
    con = np.stack([sc2, bq2, bk2, bv.reshape(ET, 128).astype(np.float32), nac, nas])
    # [6, ET, 128] -> [128, 6*ET] with column layout (kind, et)
    con = np.ascontiguousarray(con.reshape(6 * ET, 128).T).astype(np.float32)
    return xT, WqT, WkT, WvT, con


def kernel(x, Wq, bq, Wk, bk, Wv, bv, phase_bias, _trace=False):
    from concourse.bass_utils import run_bass_kernel_spmd

    x = np.asarray(x, dtype=np.float32)
    xT, WqT, WkT, WvT, con = _host_prep(
        x, np.asarray(Wq, np.float32), np.asarray(bq, np.float32),
        np.asarray(Wk, np.float32), np.asarray(bk, np.float32),
        np.asarray(Wv, np.float32), np.asarray(bv, np.float32),
        np.asarray(phase_bias, np.float32))

    if "prog" not in _cache:
        _cache["prog"] = _build_program()
    nc = _cache["prog"]

    in_maps = []
    for c in range(NCORES):
        b, qb = c // 4, c % 4
        in_maps.append({
            "xT": xT[b],
            "xTq": np.ascontiguousarray(xT[b][:, qb * QBLK : (qb + 1) * QBLK]),
            "WqT": WqT, "WkT": WkT, "WvT": WvT,
            "CON": con,
        })
    res = run_bass_kernel_spmd(nc, in_maps, core_ids=list(range(NCORES)),
                               trace=_trace)
    out = np.empty((B, S, D), dtype=np.float32)
    for c in range(NCORES):
        b, qb = c // 4, c % 4
        out[b, qb * QBLK : (qb + 1) * QBLK, :] = res.results[c]["OT"].T
    if _trace:
        kernel.last_exec_time_ns = res.exec_time_ns
        kernel.last_result = res
    return out


# revision 28
# speedup vs baseline: 1.5435x; 1.0717x over previous
"""EulerAttention Trainium2 kernel (v2: fp8 DoubleRow scores).

Per-core sharding: core c in 0..7 -> (batch b = c // 4, query block qb = c % 4,
1024 queries each).  Each core computes K features for its whole batch, Q
features for its query block, then flash-style scores/softmax/AV.

v2 design:
- All projections f32r (full-rate).  Q/K features (cos/sin theta) are stored
  as fp8e4 and the [S,S] score matmuls run in fp8 DoubleRow perf mode (2
  128-feature tiles contracted per pass, 0.5 cyc/row -> 4x f32r).
- K-side features are mean-centered per feature (host-computed E[cos theta_k],
  E[sin theta_k] from the weights); the dropped cross terms are per-query
  constants that cancel in softmax normalization.  Centering shrinks fp8
  quantization noise from the near-constant long-wavelength features.
- K features stay SBUF-resident (fp8, 64KB/partition); no DRAM roundtrip.
- V and the exp(scores) tiles are bf16; AV + rowsum matmuls run bf16.
  V stays SBUF-resident per 2-sblk AV group.  Output accumulator bf16.
- Two sweeps: sweep 1 = Q features + K features (wq/wk resident);
  sweep 2 = V proj + scores + softmax + AV (wv resident, x reloaded).

kernel(**inputs) takes the full unsharded inputs from reference.setup_inputs()
and returns the full [B, S, D] output.
"""
import sys, math

sys.path.insert(0, "/opt/trn_rl_repo")

import numpy as np
import ml_dtypes

B, S, D = 2, 4096, 1024
NCORES = 8
QBLK = S // 4          # queries per core
ET = D // 128          # number of 128-row e/d tiles (8)
NSLOT = 2 * ET         # fp8 feature slots (cos/sin interleaved per et)
MAGIC = float(1.5 * 2**23)
TWOPI = 2.0 * math.pi
INV_SQRT_D = 1.0 / math.sqrt(D)
F8NP = ml_dtypes.float8_e4m3
FP8_FROM = 2              # e-tiles >= this run the Q/K projection in fp8 DoubleRow

_cache = {}


def _build_program(trace_sim=False):
    import concourse.bass as bass
    from concourse import bacc
    import concourse.mybir as mybir
    import concourse.tile as tile
    from contextlib import ExitStack

    f32 = mybir.dt.float32
    f32r = mybir.dt.float32r
    bf16 = mybir.dt.bfloat16
    f8 = mybir.dt.float8e4
    Act = mybir.ActivationFunctionType
    Alu = mybir.AluOpType
    PM = mybir.MatmulPerfMode

    s_keys, s_q = S, QBLK
    n_sblk = s_keys // 512       # 8 key production blocks
    n_tt = s_keys // 128         # 32 key tiles
    n_qsb = s_q // 512           # 2 query production blocks
    NS = s_q                     # resident query width (1024)
    n_ns = NS // 512             # N-splits for matmuls over queries

    nc = bacc.Bacc("TRN2", target_bir_lowering=False, debug=False)

    xT = nc.dram_tensor("xT", [D, s_keys], f32r, kind="ExternalInput").ap()
    xTq = nc.dram_tensor("xTq", [D, s_q], f32r, kind="ExternalInput").ap()
    XT8 = nc.dram_tensor("XT8", [D, s_keys], f8, kind="ExternalInput").ap()
    XTQ8 = nc.dram_tensor("XTQ8", [D, s_q], f8, kind="ExternalInput").ap()
    # f32r weight slices for the low e-tiles (phase-precision-critical)
    WqT = nc.dram_tensor("WqT", [D, FP8_FROM * 128], f32r, kind="ExternalInput").ap()
    WkT = nc.dram_tensor("WkT", [D, FP8_FROM * 128], f32r, kind="ExternalInput").ap()
    WQ8 = nc.dram_tensor("WQ8", [D, D], f8, kind="ExternalInput").ap()
    WK8 = nc.dram_tensor("WK8", [D, D], f8, kind="ExternalInput").ap()
    WvT = nc.dram_tensor("WvT", [D, D], f32r, kind="ExternalInput").ap()
    # packed per-partition constants: columns = (sc2 | bq2 | bk2 | bv | nac | nas) x ET
    CON = nc.dram_tensor("CON", [128, 6 * ET], f32, kind="ExternalInput").ap()

    OT = nc.dram_tensor("OT", [D, s_q], f32, kind="ExternalOutput").ap()

    with tile.TileContext(nc, trace_sim=trace_sim) as tc, ExitStack() as top:
        # ---- constants (tiny, load first) ----
        cpool = top.enter_context(tc.tile_pool(name="consts", bufs=1))
        ctile = cpool.tile([128, 6 * ET], f32, tag="ctile")
        nc.sync.dma_start(ctile[:], CON[:])
        sc2 = [ctile[:, i : i + 1] for i in range(ET)]
        bq2 = [ctile[:, ET + i : ET + i + 1] for i in range(ET)]
        bk2 = [ctile[:, 2 * ET + i : 2 * ET + i + 1] for i in range(ET)]
        bvt = [ctile[:, 3 * ET + i : 3 * ET + i + 1] for i in range(ET)]
        nac = [ctile[:, 4 * ET + i : 4 * ET + i + 1] for i in range(ET)]
        nas = [ctile[:, 5 * ET + i : 5 * ET + i + 1] for i in range(ET)]
        ones_col = cpool.tile([128, 2], bf16, tag="ones_col")  # [K=128, M=2] rowsum lhsT
        nc.vector.memset(ones_col[:], 1.0)
        ones_row = cpool.tile([1, 128], bf16, tag="ones_row")  # [K=1, M=128] bcast lhsT
        nc.vector.memset(ones_row[:], 1.0)

        # ---- shared PSUM pool ----
        psum = top.enter_context(tc.tile_pool(name="psum", bufs=1, space="PSUM"))

        # ---- output accumulator (bf16), lives until the final normalize ----
        oacc = top.enter_context(tc.tile_pool(name="oacc", bufs=1))
        o_ac = [oacc.tile([128, NS], bf16, tag=f"o{dt}", name=f"oac{dt}")
                for dt in range(ET)]

        # ---- resident fp8 feature maps ----
        # Q: [128, slot, q]; K: per sblk [128, slot, 512 keys]
        qres_ctx = tc.tile_pool(name="qres", bufs=1)
        qres = qres_ctx.__enter__()
        qa8 = qres.tile([128, NSLOT, NS], f8, tag="qa8")

        kpool_ctx = tc.tile_pool(name="kres", bufs=1)
        kpool = kpool_ctx.__enter__()
        kres = [kpool.tile([128, NSLOT, 512], f8, tag=f"kr{s}", name=f"kr{s}")
                for s in range(n_sblk)]

        def qslot(et, cs, qsb):
            return qa8[:, 2 * et + cs, qsb * 512 : qsb * 512 + 512]

        # ================= SWEEP 1: Q + K feature maps =================
        p1_ctx = tc.tile_pool(name="p1sb", bufs=2)
        p1 = p1_ctx.__enter__()
        w1_ctx = tc.tile_pool(name="w1", bufs=1)
        w1 = w1_ctx.__enter__()
        pch_ctx = tc.tile_pool(name="p1chain", bufs=2)
        pch = pch_ctx.__enter__()

        # f32r weights for e-tiles < FP8_FROM; fp8 d-pair tiles for the rest
        wq = [w1.tile([128, FP8_FROM * 128], f32r, tag=f"wq{d}", name=f"wq{d}")
              for d in range(ET)]
        wk = [w1.tile([128, FP8_FROM * 128], f32r, tag=f"wk{d}", name=f"wk{d}")
              for d in range(ET)]
        wq8 = [w1.tile([128, 2, D], f8, tag=f"wq8{m}", name=f"wq8{m}")
               for m in range(ET // 2)]
        wk8 = [w1.tile([128, 2, D], f8, tag=f"wk8{m}", name=f"wk8{m}")
               for m in range(ET // 2)]

        def load_xblk(src_ap, col0, src8_ap=None):
            br = p1.tile([128, ET * 512], f32r, tag="xbr", name="xbr")
            nc.sync.dma_start(
                br[:].rearrange("p (d s) -> p d s", d=ET),
                src_ap[:, col0 : col0 + 512].rearrange("(d p) s -> p d s", p=128))
            if src8_ap is None:
                return br, None
            x8 = p1.tile([128, ET, 512], f8, tag="x8", name="x8")
            nc.sync.dma_start(
                x8[:],
                src8_ap[:, col0 : col0 + 512].rearrange("(d p) s -> p d s", p=128))
            return br, x8

        def theta_chain(xb, et, w_tiles, w8_tiles, bias_tiles):
            """Projection + range reduction; returns (f, g) turn tiles for Sin."""
            br, x8 = xb
            ps = psum.tile([128, 512], f32, tag="proj", name="psf", bufs=2)
            if et < FP8_FROM or x8 is None:
                for d in range(ET):
                    nc.tensor.matmul(ps[:], w_tiles[d][:, et * 128 : (et + 1) * 128],
                                     br[:, d * 512 : (d + 1) * 512],
                                     start=(d == 0), stop=(d == ET - 1))
            else:
                for m in range(ET // 2):
                    nc.tensor.matmul(ps[:],
                                     w8_tiles[m][:, :, et * 128 : (et + 1) * 128],
                                     x8[:, 2 * m : 2 * m + 2, :],
                                     start=(m == 0), stop=(m == ET // 2 - 1),
                                     perf_mode=PM.DoubleRow)
            r = pch.tile([128, 512], f32, tag="r", name="r")
            nc.scalar.activation(r[:], ps[:], Act.Identity,
                                 scale=sc2[et][:], bias=bias_tiles[et][:])
            kk = pch.tile([128, 512], f32, tag="kk", name="kk")
            nc.vector.tensor_scalar(kk[:], r[:], MAGIC, MAGIC, Alu.add, Alu.subtract)
            f = pch.tile([128, 512], f32, tag="f", name="f")
            nc.vector.scalar_tensor_tensor(f[:], kk[:], -1.0, r[:],
                                           Alu.mult, Alu.add)
            g = pch.tile([128, 512], f32, tag="kk", name="g")
            nc.vector.add_range_wrap(g[:], f[:], 0.25, 0.5, 1.0)
            return f, g

        # --- Q features: ACT Sin writes fp8 directly into qa8 ---
        for d in range(ET):
            nc.sync.dma_start(wq[d][:], WqT[d * 128 : (d + 1) * 128, :])
        xq_blocks = [load_xblk(xTq, 0, XTQ8)]
        for m in range(ET // 2):
            nc.sync.dma_start(
                wq8[m][:], WQ8[2 * m * 128 : (2 * m + 2) * 128, :]
                .rearrange("(j p) e -> p j e", p=128))
        if n_qsb > 1:
            xq_blocks.append(load_xblk(xTq, 512, XTQ8))
        for d in range(ET):
            nc.sync.dma_start(wk[d][:], WkT[d * 128 : (d + 1) * 128, :])
        for m in range(ET // 2):
            nc.sync.dma_start(
                wk8[m][:], WK8[2 * m * 128 : (2 * m + 2) * 128, :]
                .rearrange("(j p) e -> p j e", p=128))
        for qsb in range(n_qsb):
            for et in range(ET):
                f, g = theta_chain(xq_blocks[qsb], et, wq, wq8, bq2)
                nc.scalar.activation(qslot(et, 1, qsb), f[:], Act.Sin, scale=TWOPI)
                nc.scalar.activation(qslot(et, 0, qsb), g[:], Act.Sin, scale=TWOPI)

        # --- K features: ACT Sin -> f32, Pool centers + converts to fp8 ---
        for sblk in range(n_sblk):
            xkb = load_xblk(xT, sblk * 512, XT8)
            for et in range(ET):
                f, g = theta_chain(xkb, et, wk, wk8, bk2)
                s32 = pch.tile([128, 512], f32, tag="s32", name="s32")
                nc.scalar.activation(s32[:], f[:], Act.Sin, scale=TWOPI)
                nc.gpsimd.tensor_scalar(kres[sblk][:, 2 * et + 1, :], s32[:],
                                        nas[et][:], None, Alu.add)
                c32 = pch.tile([128, 512], f32, tag="s32", name="c32")
                nc.scalar.activation(c32[:], g[:], Act.Sin, scale=TWOPI)
                nc.gpsimd.tensor_scalar(kres[sblk][:, 2 * et, :], c32[:],
                                        nac[et][:], None, Alu.add)

        pch_ctx.__exit__(None, None, None)
        w1_ctx.__exit__(None, None, None)

        # ================= SWEEP 2: V proj + scores + softmax + AV =========
        with tc.tile_pool(name="w2", bufs=1) as w2, \
             tc.tile_pool(name="vres", bufs=1) as vpool, \
             tc.tile_pool(name="epool", bufs=9) as epool, \
             tc.tile_pool(name="pnorm", bufs=1) as pnorm:

            wv = [w2.tile([128, D], f32r, tag=f"wv{d}", name=f"wv{d}")
                  for d in range(ET)]
            for d in range(ET):
                nc.sync.dma_start(wv[d][:], WvT[d * 128 : (d + 1) * 128, :])

            # V resident per 2-sblk AV group: [128 p=key%128, ti, dv]
            vres = [vpool.tile([128, 4, D], bf16, tag=f"v{i}", name=f"v{i}")
                    for i in range(2)]
            ps_rs = psum.tile([2, NS], f32, tag="rs", bufs=1)

            e_group = []
            for sblk in range(n_sblk):
                xkb, _ = load_xblk(xT, sblk * 512)
                # V projection for this key block
                for ti in range(4):
                    for dg in range(2):
                        psv = psum.tile([128, 512], f32, tag="proj", name="psv",
                                        bufs=2)
                        for d in range(ET):
                            nc.tensor.matmul(
                                psv[:],
                                xkb[:, d * 512 + ti * 128 : d * 512 + (ti + 1) * 128],
                                wv[d][:, dg * 512 : dg * 512 + 512],
                                start=(d == 0), stop=(d == ET - 1))
                        nc.vector.tensor_copy(
                            vres[sblk % 2][:, ti, dg * 512 : (dg + 1) * 512], psv[:])
                # scores + exp for the 4 key tiles of this block
                for loc in range(4):
                    tt = sblk * 4 + loc
                    ps_sim = psum.tile([128, NS], f32, tag="big", name="ps_sim",
                                       bufs=2)
                    for ns in range(n_ns):
                        sl = slice(ns * 512, ns * 512 + 512)
                        for j in range(ET):
                            nc.tensor.matmul(
                                ps_sim[:, sl],
                                kres[sblk][:, 2 * j : 2 * j + 2,
                                           loc * 128 : (loc + 1) * 128],
                                qa8[:, 2 * j : 2 * j + 2, ns * 512 : ns * 512 + 512],
                                start=(j == 0), stop=(j == ET - 1),
                                perf_mode=PM.DoubleRow)
                    et_t = epool.tile([128, NS], bf16, tag="e", name="e")
                    nc.scalar.activation(et_t[:], ps_sim[:], Act.Exp,
                                         scale=INV_SQRT_D)
                    e_group.append(et_t)
                    for ns in range(n_ns):
                        sl = slice(ns * 512, ns * 512 + 512)
                        nc.tensor.matmul(ps_rs[:, sl], ones_col[:], et_t[:, sl],
                                         start=(tt == 0), stop=(tt == n_tt - 1))
                if sblk == n_sblk - 1:
                    # rowsum chain just closed: compute 1/rowsum + broadcast now
                    # so it overlaps the final AV group
                    rec = pnorm.tile([1, NS], bf16, tag="rec")
                    with nc.allow_low_precision(
                            reason="1/rowsum broadcast runs bf16; the bc tile "
                                   "it feeds is bf16 anyway"):
                        nc.vector.reciprocal(rec[:], ps_rs[:1, :])
                    ps_bc = psum.tile([128, NS], f32, tag="big", name="ps_bc",
                                      bufs=2)
                    for ns in range(n_ns):
                        sl = slice(ns * 512, ns * 512 + 512)
                        nc.tensor.matmul(ps_bc[:, sl], ones_row[:], rec[:, sl],
                                         start=True, stop=True)
                    bc = pnorm.tile([128, NS], bf16, tag="bc")
                    nc.vector.tensor_copy(bc[:], ps_bc[:])
                # AV for the group of 8 key tiles (2 sblks)
                if sblk % 2 == 1:
                    tg = sblk // 2
                    for dg in range(2):
                        for di in range(4):
                            dt = dg * 4 + di
                            ps_o = psum.tile([128, NS], f32, tag="big", name="ps_o",
                                             bufs=2)
                            for gi in range(8):
                                g_s, ti = gi // 4, gi % 4
                                for ns in range(n_ns):
                                    sl = slice(ns * 512, ns * 512 + 512)
                                    nc.tensor.matmul(
                                        ps_o[:, sl],
                                        vres[g_s][:, ti,
                                                  dt * 128 : (dt + 1) * 128],
                                        e_group[gi][:, sl],
                                        start=(gi == 0), stop=(gi == 7))
                            if tg == 0:
                                nc.vector.tensor_copy(o_ac[dt][:], ps_o[:])
                            else:
                                nc.vector.tensor_tensor(o_ac[dt][:], ps_o[:],
                                                        o_ac[dt][:], Alu.add)
                    e_group = []

            # normalize + V bias; `on` staging reuses the p1 x-block slots
            for dt in range(ET):
                on = p1.tile([128, NS], f32, tag="xbr", name="on")
                nc.vector.tensor_tensor(on[:], o_ac[dt][:], bc[:], Alu.mult)
                nc.scalar.activation(on[:], on[:], Act.Identity, bias=bvt[dt][:])
                nc.sync.dma_start(OT[dt * 128 : (dt + 1) * 128, :], on[:])

        p1_ctx.__exit__(None, None, None)
        kpool_ctx.__exit__(None, None, None)
        qres_ctx.__exit__(None, None, None)

    nc.compile()
    return nc


def _host_prep(x, Wq, bq, Wk, bk, Wv, bv, phase_bias):
    wavelengths = np.arange(1, D + 1, dtype=np.float32) * np.float32(2.0 * math.pi / D)
    inv_wl = (np.float32(1.0) / (wavelengths + np.float32(1e-8))).astype(np.float32)
    sc2 = (inv_wl / TWOPI).astype(np.float32).reshape(ET, 128)
    bq2 = ((bq * inv_wl + phase_bias) / TWOPI).astype(np.float32).reshape(ET, 128)
    bk2 = ((bk * inv_wl + phase_bias) / TWOPI).astype(np.float32).reshape(ET, 128)
    # K-feature means from the weights: theta_k ~ N(bk*ivl + pb, |wk_row|^2 ivl^2)
    mu = (bk * inv_wl + phase_bias).astype(np.float64)
    var = (np.sum(Wk.astype(np.float64) ** 2, axis=1) * inv_wl.astype(np.float64) ** 2)
    damp = np.exp(-var / 2.0)
    nac = (-(np.cos(mu) * damp)).astype(np.float32).reshape(ET, 128)
    nas = (-(np.sin(mu) * damp)).astype(np.float32).reshape(ET, 128)
    WqTf = np.ascontiguousarray(Wq.T).astype(np.float32)
    WkTf = np.ascontiguousarray(Wk.T).astype(np.float32)
    WqT = np.ascontiguousarray(WqTf[:, : FP8_FROM * 128])
    WkT = np.ascontiguousarray(WkTf[:, : FP8_FROM * 128])
    WQ8 = WqTf.astype(F8NP)
    WK8 = WkTf.astype(F8NP)
    WvT = np.ascontiguousarray(Wv.T).astype(np.float32)
    xT = [np.ascontiguousarray(x[b].T).astype(np.float32) for b in range(x.shape[0])]
    xT8 = [t.astype(F8NP) for t in xT]
    con = np.stack([sc2, bq2, bk2, bv.reshape(ET, 128).astype(np.float32), nac, nas])
    # [6, ET, 128] -> [128, 6*ET] with column layout (kind, et)
    con = np.ascontiguousarray(con.reshape(6 * ET, 128).T).astype(np.float32)
    return xT, xT8, WqT, WkT, WQ8, WK8, WvT, con


def kernel(x, Wq, bq, Wk, bk, Wv, bv, phase_bias, _trace=False):
    from concourse.bass_utils import run_bass_kernel_spmd

    x = np.asarray(x, dtype=np.float32)
    xT, xT8, WqT, WkT, WQ8, WK8, WvT, con = _host_prep(
        x, np.asarray(Wq, np.float32), np.asarray(bq, np.float32),
        np.asarray(Wk, np.float32), np.asarray(bk, np.float32),
        np.asarray(Wv, np.float32), np.asarray(bv, np.float32),
        np.asarray(phase_bias, np.float32))

    if "prog" not in _cache:
        _cache["prog"] = _build_program()
    nc = _cache["prog"]

    in_maps = []
    for c in range(NCORES):
        b, qb = c // 4, c % 4
        in_maps.append({
            "xT": xT[b],
            "xTq": np.ascontiguousarray(xT[b][:, qb * QBLK : (qb + 1) * QBLK]),
            "XT8": xT8[b],
            "XTQ8": np.ascontiguousarray(xT8[b][:, qb * QBLK : (qb + 1) * QBLK]),
            "WqT": WqT, "WkT": WkT, "WQ8": WQ8, "WK8": WK8, "WvT": WvT,
            "CON": con,
        })
    res = run_bass_kernel_spmd(nc, in_maps, core_ids=list(range(NCORES)),
                               trace=_trace)
    out = np.empty((B, S, D), dtype=np.float32)
    for c in range(NCORES):
        b, qb = c // 4, c % 4
        out[b, qb * QBLK : (qb + 1) * QBLK, :] = res.results[c]["OT"].T
    if _trace:
        kernel.last_exec_time_ns = res.exec_time_ns
        kernel.last_result = res
    return out
